# revision 1
# baseline (speedup 1.0000x reference)
"""Trainium2 Bass kernel for nn_DepthCueRectification_Sp.

Data-parallel over batch: 8 batch elements -> 8 NeuronCores (SPMD).

Per-core pipeline (D=768, N=1024, token pad NPAD=1152):
  tT    = U @ xb.T                  (bf16)
  yUT   = U @ yb.T                  (bf16)   [algebra: logits_k =
                                     (|S_k|*t) @ (y@U.T).T]
  tsT_k = |S_k|-scaled copies of tT (ACT per-partition scale)
  pos   = PE identity-accumulate of host-prescaled coord planes
          (cplw[i,j,c] = -|p|*pe[i,c]*coords[i,j,c]), exp on ACT
  logits_k -> exp (no max-sub, fused row-sum) -> attn_k = 256*attn (bf16)
  entropy: Ln on ACT; multiply+reduce on Pool (raw accum); routing
  compares raw accums; heat = 2e/(1+e), e = exp(-ht*H_sel) via one exp
  dka (selected attn, f32) -> PE transpose (f32) -> acT fp8 pairs
  y_outT = fp8 DoubleRow (ybp pairs @ acT pairs) -> yf8 = 16*y_full.T fp8
  MLP W1: x-half bf16 (xtb @ 32*W1a) + y-half fp8 DR (yf8 @ 2*W1b) = 32*h
          CLS y-half zeroed on device; exact host correction hct added.
  gel   = gelu(psh/32 + b1) -> fp8 pairs
  MLP W2: fp8 DoubleRow (gel pairs @ 32*W2 pairs) = 32*xp'
  out   = x + heat*(xp' + b2)

The act-table dict is patched so Exp and Ln resolve to the combined
natural_log_exp_and_others set (avoids per-iteration table reloads).
tensor_tensor_reduce and 16-bit PE transposes hard-crash the exec unit
on this toolchain and are not used.
"""

import os
import sys

if "/opt/trn_rl_repo" not in sys.path:
    sys.path.insert(0, "/opt/trn_rl_repo")

import numpy as np
import ml_dtypes

import concourse.bass as bass
import concourse.bass_utils as _bu
import concourse.mybir as mybir
import concourse.tile as tile
from concourse import bacc
from concourse.bass_utils import run_bass_kernel_spmd
from concourse.hw_specs import get_activation_tables
from concourse.masks import make_identity

# Enable walrus's LDWEIGHTS elision (skips redundant weight reloads when
# consecutive matmuls share a stationary operand). concourse pins it off;
# correctness is covered by the rel-err check.
if int(os.environ.get("K_LDWOPT", "0")) and not getattr(_bu, "_ldwopt_patched", False):
    _orig_run_command = _bu.run_command

    def _run_command_ldwopt(cmd, **kw):
        if cmd and "walrus_driver" in str(cmd[0]):
            cmd = [c.replace("--enable-ldw-opt=false", "--enable-ldw-opt=true")
                   if isinstance(c, str) else c for c in cmd]
        return _orig_run_command(cmd, **kw)

    _bu.run_command = _run_command_ldwopt
    _bu._ldwopt_patched = True

B, N, D, DFF, CLS = 8, 1024, 768, 3072, 1
NP1 = N + CLS          # 1025
NPAD = 1152            # 9 * 128
ND = D // 128          # 6
NB = N // 128          # 8
NF = DFF // 128        # 24
AF = mybir.ActivationFunctionType
ALU = mybir.AluOpType
dt = mybir.dt
DR = mybir.MatmulPerfMode.DoubleRow

# Note: walrus's LDW-elision pass (--enable-ldw-opt) rejects both the
# DoubleRow and transpose Ldweights this kernel emits; keep it off.
NODR = bool(int(os.environ.get("K_NODR", "0")))      # disable DoubleRow
NOPOOL = bool(int(os.environ.get("K_NOPOOL", "0")))  # entropy ops on DVE
W2BF = bool(int(os.environ.get("K_W2BF", "0")))      # W2 in bf16

SCALE = float(D) ** -0.5
SA = 256.0             # attn scale (fp8 headroom)
SY = 16.0              # y_full scale in yf8
SW1X = 32.0            # W1 x-half scale (bf16)  == SW1Y*SY
SW1Y = 2.0             # W1 y-half scale (fp8)
SW2 = 32.0             # W2 scale (fp8)
LN256 = float(np.log(SA))

_prog_cache = {}


def _patch_act_tables(arch):
    """Make natural_log_exp_and_others the only provider of Exp/Ln so the
    compiler's table-load pass keeps one table across the attention loop.
    Mutates the functools-cached dict in place (names/ids unchanged)."""
    tabs = get_activation_tables(arch)
    keep = "natural_log_exp_and_others"
    if keep not in tabs:
        return
    for name, s in tabs.items():
        if name == keep:
            continue
        s.discard(AF.Exp)
        s.discard(AF.Ln)


def _build(g, ht, pt):
    omg = 1.0 - g
    f8 = dt.float8e4
    bf = dt.bfloat16
    f32 = dt.float32

    nc = bacc.Bacc("TRN2", target_bir_lowering=False, debug=False, num_devices=8)
    _patch_act_tables(nc.m.arch)

    def mm_dr(out, l3, r3, start, stop):
        if not NODR:
            nc.tensor.matmul(out, l3, r3, start=start, stop=stop, perf_mode=DR)
        else:
            nc.tensor.matmul(out, l3[:, 0], r3[:, 0], start=start, stop=False)
            nc.tensor.matmul(out, l3[:, 1], r3[:, 1], start=False, stop=stop)

    ENT = nc.vector if NOPOOL else nc.gpsimd

    # ---- DRAM params ----
    xtb_d = nc.declare_dram_parameter("xtb", [128, ND, NPAD], bf, isOutput=False)
    yt_d = nc.declare_dram_parameter("yt", [128, ND, NP1], bf, isOutput=False)
    ybp_d = nc.declare_dram_parameter("ybp", [128, 4, 2, D], f8, isOutput=False)
    utb_d = nc.declare_dram_parameter("utb", [128, ND, ND, 128], bf, isOutput=False)
    w1p_d = nc.declare_dram_parameter("w1p", [128, 3, NF, 2, 128], f8, isOutput=False)
    w1x_d = nc.declare_dram_parameter("w1x", [128, ND, NF, 128], bf, isOutput=False)
    if W2BF:
        w2r_d = nc.declare_dram_parameter("w2r", [128, NF, D], bf, isOutput=False)
    else:
        w2r_d = nc.declare_dram_parameter("w2r", [128, 12, 2, D], f8, isOutput=False)
    b1t_d = nc.declare_dram_parameter("b1t", [128, NF], f32, isOutput=False)
    hct_d = nc.declare_dram_parameter("hct", [128, NF], f32, isOutput=False)
    b2b_d = nc.declare_dram_parameter("b2b", [128, D], f32, isOutput=False)
    s12_d = nc.declare_dram_parameter("s12", [128, 2, ND], f32, isOutput=False)
    cpl_d = nc.declare_dram_parameter("cpl", [NB, 128, 6, N], bf, isOutput=False)
    xnat_d = nc.declare_dram_parameter("xnat", [NPAD, D], f32, isOutput=False)
    out_d = nc.declare_dram_parameter("out", [NPAD, D], f32, isOutput=True)
    hmbuf = nc.dram_tensor("hmbuf", [NPAD, 1], f32)

    with tile.TileContext(nc) as tc:
        with tc.tile_pool(name="p0", bufs=1) as P0:
            # ---- persistent tiles ----
            w1p = P0.tile([128, 3, NF, 2, 128], f8, tag="w1p", name="w1p")
            w1x = P0.tile([128, ND, NF, 128], bf, tag="w1x", name="w1x")
            xtb = P0.tile([128, ND, NPAD], bf, tag="xtb", name="xtb")
            yf8 = P0.tile([128, 3, 2, NPAD], f8, tag="yf8", name="yf8")
            b2b = P0.tile([128, D], f32, tag="b2b", name="b2b")
            b1t = P0.tile([128, NF], f32, tag="b1t", name="b1t")
            hct = P0.tile([128, NF], f32, tag="hct", name="hct")
            s12 = P0.tile([128, 2, ND], f32, tag="s12", name="s12")
            identf = P0.tile([128, 128], f32, tag="identf", name="identf")
            identm = P0.tile([128, 128], bf, tag="identm", name="identm")
            epsb = P0.tile([128, 1], f32, tag="epsb", name="epsb")
            onep = P0.tile([1, 1], f32, tag="onep", name="onep")
            zerop = P0.tile([128, 1], f32, tag="zerop", name="zerop")
            hbias = P0.tile([128, 1], f32, tag="hbias", name="hbias")

            # ---- gpsimd queue: small inits, then the big weight loads ----
            nc.gpsimd.dma_start(s12[:], s12_d[:])
            make_identity(nc, identf[:])
            make_identity(nc, identm[:])
            nc.gpsimd.memset(epsb[:], SA * 1e-8)
            nc.gpsimd.memset(hbias[:], -ht * LN256)
            nc.gpsimd.memset(onep[:], 1.0)
            nc.gpsimd.memset(zerop[:], 0.0)
            nc.gpsimd.memset(yf8[:, :, :, NP1:NPAD], 0.0)
            nc.gpsimd.memset(yf8[:, :, :, 0:CLS], 0.0)
            nc.gpsimd.dma_start(hmbuf[0:1, 0:1], onep[:])
            nc.gpsimd.dma_start(hmbuf[NP1:NPAD, 0:1], zerop[0 : NPAD - NP1, 0:1])
            for yp in range(3):
                nc.gpsimd.dma_start(w1p[:, yp], w1p_d[:, yp])
            for c in range(ND):
                nc.gpsimd.dma_start(w1x[:, c], w1x_d[:, c])

            # ---- scalar queue: small consts ----
            nc.scalar.dma_start(b2b[:], b2b_d[:])
            nc.scalar.dma_start(b1t[:], b1t_d[:])
            nc.scalar.dma_start(hct[:], hct_d[:])

            # ---- PE warmup: ramp the tensor engine to full clock while
            # the input DMAs stream (identm has no DMA dependency). ----
            with tc.tile_pool(name="pwu", bufs=1, space="PSUM") as PWU:
                wps = PWU.tile([128, 128], f32, tag="wps", name="wps")
                for _ in range(24):
                    nc.tensor.matmul(wps[:], identm[:], identm[:],
                                     start=True, stop=True)

            with tc.tile_pool(name="pa2", bufs=1) as PA2:
                acT = PA2.tile([128, 4, 2, N], f8, tag="acT", name="acT")
                ybp = PA2.tile([128, 4, 2, D], f8, tag="ybp", name="ybp")

                with tc.tile_pool(name="pa1", bufs=1) as PA1:
                    yUT = PA1.tile([128, ND, N], bf, tag="yUT", name="yUT")
                    ts0 = PA1.tile([128, ND, N], bf, tag="ts0", name="ts0")
                    ts1 = PA1.tile([128, ND, N], bf, tag="ts1", name="ts1")
                    posn = PA1.tile([128, NB, N], bf, tag="posn", name="posn")

                    # ---------- phase 1: tT, yUT, pos ----------
                    with tc.tile_pool(name="p1", bufs=1) as P1, \
                         tc.tile_pool(name="ps1", bufs=2, space="PSUM") as PS1:
                        utb = P1.tile([128, ND, ND, 128], bf, tag="utb", name="utb")
                        yt = P1.tile([128, ND, NP1], bf, tag="yt", name="yt")
                        for d in range(ND):
                            nc.scalar.dma_start(utb[:, d], utb_d[:, d])
                        for k in range(ND):
                            nc.sync.dma_start(xtb[:, k], xtb_d[:, k])
                        for k in range(ND):
                            nc.scalar.dma_start(yt[:, k], yt_d[:, k])

                        for d in range(ND):
                            ps = PS1.tile([128, N], f32, tag="psA", name="psA")
                            for k in range(ND):
                                for h in range(2):
                                    nc.tensor.matmul(
                                        ps[:, 512 * h : 512 * h + 512],
                                        utb[:, d, k],
                                        xtb[:, k, CLS + 512 * h : CLS + 512 * h + 512],
                                        start=(k == 0), stop=(k == ND - 1),
                                    )
                            nc.scalar.mul(ts0[:, d, :], ps[:], s12[:, 0, d : d + 1])
                            nc.scalar.mul(ts1[:, d, :], ps[:], s12[:, 1, d : d + 1])
                        for d in range(ND):
                            ps = PS1.tile([128, N], f32, tag="psA", name="psA")
                            for k in range(ND):
                                for h in range(2):
                                    nc.tensor.matmul(
                                        ps[:, 512 * h : 512 * h + 512],
                                        utb[:, d, k],
                                        yt[:, k, CLS + 512 * h : CLS + 512 * h + 512],
                                        start=(k == 0), stop=(k == ND - 1),
                                    )
                            nc.scalar.copy(yUT[:, d, :], ps[:])

                    # ---- phase 2: pos (PE-accumulated) interleaved with
                    #      attention, entropy, routing ----
                    with tc.tile_pool(name="pcp", bufs=2) as CPP, \
                         tc.tile_pool(name="ppo", bufs=2) as PO, \
                         tc.tile_pool(name="psm0", bufs=4) as SM0, \
                         tc.tile_pool(name="pat", bufs=3) as PT, \
                         tc.tile_pool(name="plk", bufs=2) as LK, \
                         tc.tile_pool(name="pdk", bufs=2) as DK, \
                         tc.tile_pool(name="psm", bufs=8) as SM, \
                         tc.tile_pool(name="psp", bufs=1, space="PSUM") as PSP, \
                         tc.tile_pool(name="psl", bufs=2, space="PSUM") as PSL, \
                         tc.tile_pool(name="pstp", bufs=2, space="PSUM") as PST:

                        def emit_pos(nb):
                            cpt = CPP.tile([128, 6, N], bf, tag="cpt", name="cpt")
                            nc.sync.dma_start(cpt[:], cpl_d[nb])
                            psp = PSP.tile([128, N], f32, tag="psp", name="psp")
                            for c in range(6):
                                for h in range(2):
                                    nc.tensor.matmul(
                                        psp[:, 512 * h : 512 * h + 512],
                                        identm[:],
                                        cpt[:, c, 512 * h : 512 * h + 512],
                                        start=(c == 0), stop=(c == 5),
                                    )
                            pxp = PO.tile([128, N], bf, tag="pxp", name="pxp")
                            pss = SM0.tile([128, 1], f32, tag="pss", name="pss")
                            nc.scalar.activation(pxp[:], psp[:], AF.Exp,
                                                 bias=zerop[:], accum_out=pss[:])
                            prg = SM0.tile([128, 1], f32, tag="prg", name="prg")
                            nc.vector.reciprocal(prg[:], pss[:])
                            nc.vector.tensor_scalar_mul(prg[:], prg[:], SA * g)
                            nc.vector.tensor_scalar_mul(
                                posn[:, nb, :], pxp[:], prg[:])

                        emit_pos(0)
                        emit_pos(1)
                        for nb in range(NB):
                            if nb == 2:
                                for mbp in range(4):
                                    nc.gpsimd.dma_start(ybp[:, mbp], ybp_d[:, mbp])
                            r0 = 128 * nb
                            pk = PT.tile([128, 2, N], bf, tag="pk", name="pk")
                            lnk = LK.tile([128, 2, N], bf, tag="lnk", name="lnk")
                            accr = SM.tile([128, 2], f32, tag="accr", name="accr")
                            for k2 in range(2):
                                tsk = ts0 if k2 == 0 else ts1
                                psl = PSL.tile([128, N], f32, tag="psl", name="psl")
                                for e in range(ND):
                                    for h in range(2):
                                        nc.tensor.matmul(
                                            psl[:, 512 * h : 512 * h + 512],
                                            tsk[:, e, r0 : r0 + 128],
                                            yUT[:, e, 512 * h : 512 * h + 512],
                                            start=(e == 0), stop=(e == ND - 1),
                                        )
                                patch = pk[:, k2, :]
                                esum = SM.tile([128, 1], f32, tag="esum", name="esum")
                                nc.scalar.activation(patch, psl[:], AF.Exp,
                                                     bias=zerop[:], scale=SCALE,
                                                     accum_out=esum[:])
                                rk = SM.tile([128, 1], f32, tag="rk", name="rk")
                                nc.vector.reciprocal(rk[:], esum[:])
                                nc.vector.tensor_scalar_mul(rk[:], rk[:], SA * omg)
                                nc.vector.scalar_tensor_tensor(
                                    patch, patch, rk[:], posn[:, nb, :],
                                    ALU.mult, ALU.add)
                                nc.scalar.activation(lnk[:, k2, :], patch, AF.Ln,
                                                     bias=epsb[:])
                            if 2 + nb < NB:
                                emit_pos(2 + nb)
                            # raw accum: accr_k = sum(attn_s * ln attn_s)
                            #          = 256*(ln256 - H_k)  (decreasing in H)
                            ENT.tensor_mul(lnk[:], lnk[:], pk[:])
                            nc.vector.tensor_reduce(
                                accr[:], lnk[:], axis=mybir.AxisListType.X,
                                op=ALU.add)
                            # route0 iff H0<=H1 iff accr0>=accr1
                            rsel = SM.tile([128, 1], f32, tag="rsel", name="rsel")
                            nc.vector.tensor_tensor(rsel[:], accr[:, 0:1],
                                                    accr[:, 1:2], ALU.is_ge)
                            amax = SM.tile([128, 1], f32, tag="amax", name="amax")
                            nc.vector.tensor_tensor(amax[:], accr[:, 0:1],
                                                    accr[:, 1:2], ALU.max)
                            # e = exp(-ht*H_sel) = exp(ht/256*amax - ht*ln256)
                            ee = SM.tile([128, 1], f32, tag="ee", name="ee")
                            nc.scalar.activation(ee[:], amax[:], AF.Exp,
                                                 scale=ht / SA, bias=hbias[:])
                            ep1 = SM.tile([128, 1], f32, tag="ep1", name="ep1")
                            nc.vector.tensor_scalar_add(ep1[:], ee[:], 1.0)
                            rcp = SM.tile([128, 1], f32, tag="rcp", name="rcp")
                            nc.vector.reciprocal(rcp[:], ep1[:])
                            heat = SM.tile([128, 1], f32, tag="heat", name="heat")
                            nc.vector.scalar_tensor_tensor(
                                heat[:], ee[:], 2.0, rcp[:], ALU.mult, ALU.mult)
                            nc.sync.dma_start(
                                hmbuf[CLS + r0 : CLS + r0 + 128, 0:1], heat[:])
                            d01 = DK.tile([128, N], bf, tag="d01", name="d01")
                            nc.vector.tensor_sub(d01[:], pk[:, 0, :], pk[:, 1, :])
                            dka = DK.tile([128, N], f32, tag="dka", name="dka")
                            nc.vector.scalar_tensor_tensor(
                                dka[:], d01[:], rsel[:], pk[:, 1, :],
                                ALU.mult, ALU.add)
                            for mb in range(NB):
                                pst = PST.tile([128, 128], f32, tag="pst", name="pst")
                                nc.tensor.transpose(
                                    pst[:], dka[:, 128 * mb : 128 * mb + 128],
                                    identf[:])
                                dst = acT[:, mb // 2, mb % 2, r0 : r0 + 128]
                                nc.scalar.copy(dst, pst[:])

                # ---------- phase 3: y_outT (fp8 DoubleRow) -> yf8 ----------
                with tc.tile_pool(name="psy", bufs=2, space="PSUM") as PSY:
                    for d in range(ND):
                        psy = PSY.tile([128, N], f32, tag="psy", name="psy")
                        if NODR:
                            for mbp in range(4):
                                for j in range(2):
                                    for h in range(2):
                                        nc.tensor.matmul(
                                            psy[:, 512 * h : 512 * h + 512],
                                            ybp[:, mbp, j, 128 * d : 128 * d + 128],
                                            acT[:, mbp, j, 512 * h : 512 * h + 512],
                                            start=(mbp == 0 and j == 0),
                                            stop=(mbp == 3 and j == 1),
                                        )
                        else:
                            for mbp in range(4):
                                for h in range(2):
                                    mm_dr(
                                        psy[:, 512 * h : 512 * h + 512],
                                        ybp[:, mbp, :, 128 * d : 128 * d + 128],
                                        acT[:, mbp, :, 512 * h : 512 * h + 512],
                                        (mbp == 0), (mbp == 3),
                                    )
                        nc.scalar.mul(yf8[:, d // 2, d % 2, CLS : CLS + N],
                                      psy[:], SY / SA)

            # ---------- phase 4: MLP ----------
            with tc.tile_pool(name="pg", bufs=1) as PG:
                if W2BF:
                    w2r = PG.tile([128, NF, D], bf, tag="w2r", name="w2r")
                    gel = PG.tile([128, NF, NPAD], bf, tag="gel", name="gel")
                else:
                    w2r = PG.tile([128, 12, 2, D], f8, tag="w2r", name="w2r")
                    gel = PG.tile([128, 12, 2, NPAD], f8, tag="gel", name="gel")
                w2ch = 6 if W2BF else 3
                for q in range(4):
                    nc.sync.dma_start(w2r[:, w2ch * q : w2ch * q + w2ch],
                                      w2r_d[:, w2ch * q : w2ch * q + w2ch])
                # pad token columns of gel are never computed; zero once so
                # the tb=8 W2 stationary reads are NaN-free
                if W2BF:
                    nc.gpsimd.memset(gel[:, :, NP1:NPAD], 0.0)
                else:
                    nc.gpsimd.memset(gel[:, :, :, NP1:NPAD], 0.0)

                # tokens 0..1023 in two 512 chunks; token 1024 (last) alone
                chunksA = [(1024, 1), (0, 512), (512, 512)]
                with tc.tile_pool(name="psh", bufs=2, space="PSUM") as PSH:
                    for f in range(NF):
                        psh = PSH.tile([128, NPAD], f32, tag="psh", name="psh")
                        for c in range(ND):
                            for (s0, wd) in chunksA:
                                nc.tensor.matmul(
                                    psh[:, s0 : s0 + wd],
                                    w1x[:, c, f],
                                    xtb[:, c, s0 : s0 + wd],
                                    start=(c == 0), stop=False,
                                )
                        if NODR:
                            for yp in range(3):
                                for j in range(2):
                                    for (s0, wd) in chunksA:
                                        nc.tensor.matmul(
                                            psh[:, s0 : s0 + wd],
                                            w1p[:, yp, f, j],
                                            yf8[:, yp, j, s0 : s0 + wd],
                                            start=False,
                                            stop=(yp == 2 and j == 1),
                                        )
                        else:
                            for yp in range(3):
                                for (s0, wd) in chunksA:
                                    mm_dr(
                                        psh[:, s0 : s0 + wd],
                                        w1p[:, yp, f],
                                        yf8[:, yp, :, s0 : s0 + wd],
                                        False, (yp == 2),
                                    )
                        # exact CLS-token y-half correction (host-computed)
                        nc.vector.tensor_add(psh[:, 0:CLS], psh[:, 0:CLS],
                                             hct[:, f : f + 1])
                        gdst = (gel[:, f, 0:NP1] if W2BF
                                else gel[:, f // 2, f % 2, 0:NP1])
                        nc.scalar.activation(gdst, psh[:, 0:NP1],
                                             AF.Gelu, bias=b1t[:, f : f + 1],
                                             scale=1.0 / SW1X)

                with tc.tile_pool(name="p5", bufs=3) as P5, \
                     tc.tile_pool(name="pso", bufs=2, space="PSUM") as PSO:
                    chunksB = [(512, D - 512), (0, 512)]
                    for tb in range(NPAD // 128):
                        r0 = 128 * tb
                        nrows = min(128, NP1 - r0)
                        if nrows <= 0:
                            continue
                        xn = P5.tile([128, D], f32, tag="xn", name="xn")
                        nc.sync.dma_start(xn[:nrows, :], xnat_d[r0 : r0 + nrows, :])
                        hmc = P5.tile([128, 1], f32, tag="hmc", name="hmc")
                        nc.sync.dma_start(hmc[:nrows, :],
                                          hmbuf[r0 : r0 + nrows, 0:1])
                        pso = PSO.tile([128, D], f32, tag="pso", name="pso")
                        if W2BF:
                            for f in range(NF):
                                for (s0, wd) in chunksB:
                                    nc.tensor.matmul(
                                        pso[:, s0 : s0 + wd],
                                        gel[:, f, r0 : r0 + 128],
                                        w2r[:, f, s0 : s0 + wd],
                                        start=(f == 0), stop=(f == NF - 1),
                                    )
                        elif NODR:
                            for fp in range(12):
                                for j in range(2):
                                    for (s0, wd) in chunksB:
                                        nc.tensor.matmul(
                                            pso[:, s0 : s0 + wd],
                                            gel[:, fp, j, r0 : r0 + 128],
                                            w2r[:, fp, j, s0 : s0 + wd],
                                            start=(fp == 0 and j == 0),
                                            stop=(fp == 11 and j == 1),
                                        )
                        else:
                            for fp in range(12):
                                for (s0, wd) in chunksB:
                                    mm_dr(
                                        pso[:, s0 : s0 + wd],
                                        gel[:, fp, :, r0 : r0 + 128],
                                        w2r[:, fp, :, s0 : s0 + wd],
                                        (fp == 0), (fp == 11),
                                    )
                        st = P5.tile([128, D], f32, tag="st", name="st")
                        if W2BF:
                            nc.vector.tensor_add(st[:], pso[:], b2b[:])
                        else:
                            nc.vector.scalar_tensor_tensor(
                                st[:], pso[:], 1.0 / SW2, b2b[:],
                                ALU.mult, ALU.add)
                        ot = P5.tile([128, D], f32, tag="ot", name="ot")
                        nc.vector.scalar_tensor_tensor(
                            ot[:nrows, :], st[:nrows, :], hmc[:nrows, :],
                            xn[:nrows, :], ALU.mult, ALU.add)
                        nc.sync.dma_start(out_d[r0 : r0 + nrows, :], ot[:nrows, :])

    nc.compile()
    return nc


def _get_prog(g, ht, pt):
    key = (round(float(g), 9), round(float(ht), 9), round(float(pt), 9))
    if key not in _prog_cache:
        _prog_cache[key] = _build(*key)
    return _prog_cache[key]


def kernel(x, y, coords, U, S1, S2, gating, h_temp, p_temp, pos_emb, W1, b1, W2, b2):
    x = np.asarray(x, dtype=np.float32)
    y = np.asarray(y, dtype=np.float32)
    coords = np.asarray(coords, dtype=np.float32)
    U = np.asarray(U, dtype=np.float32)
    bf16 = ml_dtypes.bfloat16
    f8 = ml_dtypes.float8_e4m3

    g = float(1.0 / (1.0 + np.exp(-float(np.asarray(gating)))))
    ht = float(np.asarray(h_temp))
    pt = abs(float(np.asarray(p_temp)))
    nc = _get_prog(g, ht, pt)

    def q8(a):
        return np.clip(a, -240.0, 240.0).astype(f8)

    # ---- shared (replicated) host prep ----
    UT = np.ascontiguousarray(U.T)
    utb = np.ascontiguousarray(
        UT.reshape(ND, 128, ND, 128).transpose(1, 2, 0, 3)).astype(bf16)
    s12 = np.ascontiguousarray(np.stack(
        [np.abs(np.asarray(S1, np.float32)).reshape(ND, 128).T,
         np.abs(np.asarray(S2, np.float32)).reshape(ND, 128).T], axis=1))
    # coords planes prescaled by -|p|*pos_emb[i,c]:
    #   cpl[nb,p,c,j] = -pt*pe[128nb+p,c] * coords[128nb+p,j,c]
    pe_f = (-pt) * np.asarray(pos_emb, np.float32)[:, :, 0]   # [N, 6]
    cplw = coords.transpose(0, 2, 1) * pe_f[:, :, None]        # [N, 6, N]
    cpl = np.ascontiguousarray(
        cplw.reshape(NB, 128, 6, N)).astype(bf16)
    W1 = np.asarray(W1, np.float32)
    W1a, W1b = W1[:D], W1[D:]
    w1x = np.ascontiguousarray(
        (SW1X * W1a).reshape(ND, 128, NF, 128).transpose(1, 0, 2, 3)).astype(bf16)
    w1p = q8((SW1Y * W1b).reshape(3, 2, 128, NF, 128).transpose(2, 0, 3, 1, 4))
    W2 = np.asarray(W2, np.float32)
    if W2BF:
        w2r = np.ascontiguousarray(
            W2.reshape(NF, 128, D).transpose(1, 0, 2)).astype(bf16)
    else:
        w2r = q8((SW2 * W2).reshape(12, 2, 128, D).transpose(2, 0, 1, 3))
    b1t = np.ascontiguousarray(np.asarray(b1, np.float32).reshape(NF, 128).T)
    b2b = np.broadcast_to(np.asarray(b2, np.float32), (128, D)).copy()

    shared = {"utb": utb, "s12": s12, "cpl": cpl,
              "w1x": w1x, "w1p": w1p, "w2r": w2r, "b1t": b1t, "b2b": b2b}

    in_maps = []
    for b in range(B):
        xp = np.zeros((NPAD, D), np.float32)
        xp[:NP1] = x[b]
        xtb = np.ascontiguousarray(
            xp.T.reshape(ND, 128, NPAD).transpose(1, 0, 2)).astype(bf16)
        yt = np.ascontiguousarray(
            y[b].T.reshape(ND, 128, NP1).transpose(1, 0, 2)).astype(bf16)
        ybp = q8(y[b, CLS:].reshape(4, 2, 128, D).transpose(2, 0, 1, 3))
        xnat = np.zeros((NPAD, D), np.float32)
        xnat[:NP1] = x[b]
        # exact CLS y-half: 32*h_y[cls] = SW1X * (y_cls @ W1b)
        hc = SW1X * (y[b, 0] @ W1b)
        hct = np.ascontiguousarray(hc.reshape(NF, 128).T)
        m = dict(shared)
        m["xtb"] = xtb
        m["yt"] = yt
        m["ybp"] = ybp
        m["xnat"] = xnat
        m["hct"] = hct
        in_maps.append(m)

    res = run_bass_kernel_spmd(nc, in_maps, list(range(B)))
    out = np.stack([res.results[b]["out"][:NP1, :] for b in range(B)])
    return out.astype(np.float32)


if __name__ == "__main__":
    import time
    sys.path.insert(0, "/root/problem")
    from reference import setup_inputs, reference

    inp = {k: np.asarray(v) for k, v in setup_inputs().items()}
    t0 = time.time()
    got = kernel(**inp)
    print("kernel wall:", time.time() - t0)
    exp = np.asarray(reference(**inp))
    d = np.abs(got - exp)
    print("absmax_rel:", d.max() / np.abs(exp).max())
    print("rms_rel:", np.sqrt((d ** 2).mean()) / np.sqrt((exp ** 2).mean()))

